# revision 25
# baseline (speedup 1.0000x reference)
"""Trainium2 Bass kernel: MeanHinAggregator (GNN message passing).

Reference computation (per batch-head element bh):
    z_r  = mean_n(x_neigh_r[bh, n, :]) @ w_neigh_r          (r = 0, 1)
    out  = relu(concat(x_self[bh] @ w_self, (z0 + z1) / 2) + b)

Strategy (pure data parallel over 8 NeuronCores, batch axis sharded):
  * The 2e-2 relative-error budget admits aggressive mixed precision.
    x_self / weights / output are bf16; one neighbour tensor (xn1) is
    cast to fp8-e4m3 on the host - its mean over 32 samples averages the
    quantization noise down, and the self projection dominates the output
    norm.  Measured end-to-end rel-err vs the fp32 reference: ~4e-3.
    Per-core HBM traffic drops 44.2 -> 16.7 MB.  (Going fp8 on BOTH
    neighbour tensors is a net loss: fp8 reads run the DVE folds at 1x -
    no 8-bit packing on TRN2 - so the kernel turns compute-bound.)
  * Per core: B_shard=128, H=10 -> 1280 rows, processed in 10 groups of
    128 rows.  Three dram tensors: xn0 bf16 [1280, 4096], xn1 fp8
    [1280, 4096], and x_self^T bf16 [128, 1280] (host-pre-transposed so
    the self projection reads it directly as lhsT - no transpose matmul,
    PSUM block, or copy; it is loaded once as a single 0.33 MB DMA).
    Each group issues two ~0.5 MiB DMAs per HWDGE ring, byte-balanced
    across rings, t1 pieces first so the slower fp8 fold starts
    earliest.  Output stores ride the ACT ring right after ReLU.
  * Mean over the 32 neighbour slices: in-place bf16 tree-folds on the
    Vector engine for xn0 (2x_1P DVE mode) and an fp8->bf16 first level
    plus a bf16 level for xn1, each down to 8 slices; eight accumulating
    transposing matmuls per tensor (lhsT = slice, rhs = identity) finish
    the sum on the PE while transposing into the [f, bh] layout the
    projection needs as lhsT.  This balances DVE (~4.9us) against DMA
    (~5us) and PE (~4us) per group.
  * Projection: out[bh, d] = sumT.T @ w with the 1/(N*NR) scaling folded
    into host-prescaled bf16 copies of w_neigh_*.  Bias is added with a
    K=1 matmul accumulating into PSUM.  PSUM -> SBUF copies are split per
    128-column block on the Scalar engine; the xn0-dependent projection
    matmul (its folds finish last) closes each group.  The first and
    last groups fold each DMA'd half independently, shortening the
    pipeline ramp and the post-last-DMA serial tail.
"""

import numpy as np
import ml_dtypes

import concourse.bacc as bacc
import concourse.bass as bass
import concourse.tile as tile
from concourse import bass_utils, mybir
from concourse._compat import with_exitstack

B, H, N, F = 1024, 10, 32, 128
HALF = 128
D = 2 * HALF
NR = 2
NCORES = 8
BSH = B // NCORES        # 128 batch rows per core
BH = BSH * H             # 1280 (bh rows per core)
GROUP = 128              # bh rows per group
NG = BH // GROUP         # 10 groups
NF = N * F               # 4096
LOOKAHEAD = 4            # groups of DMA prefetch beyond the current one
F32 = mybir.dt.float32
BF16 = mybir.dt.bfloat16
FP8 = mybir.dt.float8e4
BF16NP = np.dtype(ml_dtypes.bfloat16)
FP8NP = np.dtype(ml_dtypes.float8_e4m3)
RELU = mybir.ActivationFunctionType.Relu
COPY = mybir.ActivationFunctionType.Copy


@with_exitstack
def _tile_kernel(ctx, tc, outs, ins, ngroups):
    nc = tc.nc
    t0_d, t1_d, xst_d, w_s, w0, w1, bvec, ident_d, ones_d = ins
    (out_d,) = outs

    const = ctx.enter_context(tc.tile_pool(name="const", bufs=1))
    xpool = ctx.enter_context(tc.tile_pool(name="xp", bufs=LOOKAHEAD + 2))
    fpool = ctx.enter_context(tc.tile_pool(name="fp", bufs=5))
    spool = ctx.enter_context(tc.tile_pool(name="sp", bufs=4))
    opool = ctx.enter_context(tc.tile_pool(name="op", bufs=4))
    ppool = ctx.enter_context(tc.tile_pool(name="ps", bufs=3, space="PSUM"))
    pout = ctx.enter_context(tc.tile_pool(name="po", bufs=3, space="PSUM"))

    def issue_loads(g):
        r = slice(g * GROUP, (g + 1) * GROUP)
        t0 = xpool.tile([128, NF], BF16, tag="t0")
        t1 = xpool.tile([128, NF], FP8, tag="t1")
        nc.sync.dma_start(t1[:, 0:2048], t1_d[r, 0:2048])
        nc.scalar.dma_start(t1[:, 2048:NF], t1_d[r, 2048:NF])
        # Byte-balanced ring split (t1 is fp8, half the bytes of t0); the
        # edge groups stay half-aligned for their per-half folds.
        cut = 2048 if (g == 0 or g == ngroups - 1) else 2176
        nc.sync.dma_start(t0[:, 0:cut], t0_d[r, 0:cut])
        nc.scalar.dma_start(t0[:, cut:NF], t0_d[r, cut:NF])
        return t0, t1

    pending = [issue_loads(0)]

    # All of x_self^T for this core in one contiguous load (2.5 KiB per
    # partition); the per-group projection just slices its columns.
    xst = const.tile([128, BH], BF16, tag="xst")
    nc.sync.dma_start(xst[:], xst_d[:])

    ident = const.tile([128, 128], BF16, tag="ident")
    nc.sync.dma_start(ident[:], ident_d[:])
    wS_t = const.tile([128, HALF], BF16, tag="wS")
    nc.sync.dma_start(wS_t[:], w_s[:])
    w0_t = const.tile([128, HALF], BF16, tag="w0")
    nc.sync.dma_start(w0_t[:], w0[:])
    w1_t = const.tile([128, HALF], BF16, tag="w1")
    nc.sync.dma_start(w1_t[:], w1[:])
    b_t = const.tile([1, D], BF16, tag="b")
    nc.sync.dma_start(b_t[:], bvec[:])
    ones_t = const.tile([1, 128], BF16, tag="ones")
    nc.sync.dma_start(ones_t[:], ones_d[:])

    for g in range(1, min(LOOKAHEAD, ngroups)):
        pending.append(issue_loads(g))

    def transpose_accum(pacc, col, f, slices):
        for i, c in enumerate(slices):
            nc.tensor.matmul(pacc[:, col:col + 128], f[:, c:c + F], ident[:],
                             start=(i == 0), stop=(i == len(slices) - 1))

    SL8 = tuple(i * F for i in range(8))

    for g in range(ngroups):
        r = slice(g * GROUP, (g + 1) * GROUP)
        t0, t1 = pending.pop(0)
        if g + LOOKAHEAD < ngroups:
            pending.append(issue_loads(g + LOOKAHEAD))
        edge = g == 0 or g == ngroups - 1
        last = g == ngroups - 1

        # Self half first: depends only on preloaded xsT/wS, so its
        # projection, ReLU and store complete long before the folds and
        # never sit on the end-of-kernel drain chain.
        po = pout.tile([128, D], F32, tag="po")
        nc.tensor.matmul(po[:, 0:HALF], ones_t[:], b_t[:, 0:HALF],
                         start=True, stop=False)
        nc.tensor.matmul(po[:, 0:HALF], xst[:, r], wS_t[:],
                         start=False, stop=True)
        ob = opool.tile([128, D], BF16, tag="ob")
        nc.scalar.activation(ob[:, 0:HALF], po[:, 0:HALF], RELU)
        nc.sync.dma_start(out_d[r, 0:HALF], ob[:, 0:HALF])

        # xn1: fp8 first level (1x on DVE) into a bf16 tile, one bf16
        # level, leaving 8 slices.  The first and last groups fold each
        # DMA'd half independently (halves land on different rings), so
        # the ramp starts earlier and the serial tail is shorter.
        f1 = fpool.tile([128, 2048], BF16, tag="f1")
        if edge:
            nc.vector.tensor_add(f1[:, 0:1024], t1[:, 0:1024],
                                 t1[:, 1024:2048])
            nc.vector.tensor_add(f1[:, 1024:2048], t1[:, 2048:3072],
                                 t1[:, 3072:NF])
            nc.vector.tensor_add(f1[:, 0:512], f1[:, 0:512], f1[:, 512:1024])
            nc.vector.tensor_add(f1[:, 1024:1536], f1[:, 1024:1536],
                                 f1[:, 1536:2048])
            nc.vector.tensor_add(f1[:, 0:256], f1[:, 0:256], f1[:, 256:512])
            nc.vector.tensor_add(f1[:, 1024:1280], f1[:, 1024:1280],
                                 f1[:, 1280:1536])
        else:
            nc.vector.tensor_add(f1[:], t1[:, 0:2048], t1[:, 2048:NF])
            nc.vector.tensor_add(f1[:, 0:1024], f1[:, 0:1024],
                                 f1[:, 1024:2048])

        # xn0: in-place bf16 levels (2x_1P), leaving 8 slices.
        if edge:
            nc.vector.tensor_add(t0[:, 0:1024], t0[:, 0:1024],
                                 t0[:, 1024:2048])
            nc.vector.tensor_add(t0[:, 2048:3072], t0[:, 2048:3072],
                                 t0[:, 3072:NF])
            nc.vector.tensor_add(t0[:, 0:512], t0[:, 0:512], t0[:, 512:1024])
            nc.vector.tensor_add(t0[:, 2048:2560], t0[:, 2048:2560],
                                 t0[:, 2560:3072])
            nc.vector.tensor_add(t0[:, 0:256], t0[:, 0:256], t0[:, 256:512])
            nc.vector.tensor_add(t0[:, 2048:2304], t0[:, 2048:2304],
                                 t0[:, 2304:2560])
        else:
            nc.vector.tensor_add(t0[:, 0:2048], t0[:, 0:2048], t0[:, 2048:NF])
            nc.vector.tensor_add(t0[:, 0:1024], t0[:, 0:1024],
                                 t0[:, 1024:2048])

        sl1 = (0, F, 1024, 1024 + F) if edge else SL8
        sl0 = (0, F, 2048, 2048 + F) if edge else SL8

        # pacc[:, 0:128] = sum_n xn0 (as [f, bh]), [:, 128:256] = sum_n
        # xn1; accumulating transposing matmuls.  x_self needs no
        # transpose: it is DMA'd pre-transposed ([f, bh]) from the host.
        pacc = ppool.tile([128, 2 * 128], F32, tag="pacc")
        transpose_accum(pacc, 128, f1, sl1)
        transpose_accum(pacc, 0, t0, sl0)

        # PSUM -> SBUF (bf16): xn1 block first, xn0 block (whose folds
        # finish last) second, so early projections don't wait.  The last
        # group runs its copies on the by-then-idle Vector engine so the
        # tail does not queue behind Scalar's earlier-group work.
        sacc = spool.tile([128, 2 * 128], BF16, tag="sacc")
        if last:
            nc.vector.tensor_copy(sacc[:, 128:256], pacc[:, 128:256])
            nc.vector.tensor_copy(sacc[:, 0:128], pacc[:, 0:128])
        else:
            nc.scalar.activation(sacc[:, 128:256], pacc[:, 128:256], COPY)
            nc.scalar.activation(sacc[:, 0:128], pacc[:, 0:128], COPY)

        # Projection: out[bh, d]; bias broadcast via K=1 matmuls; the
        # xn0-dependent matmul (its folds finish last) closes each group.
        nc.tensor.matmul(po[:, HALF:D], ones_t[:], b_t[:, HALF:D],
                         start=True, stop=False)
        nc.tensor.matmul(po[:, HALF:D], sacc[:, 128:256], w1_t[:],
                         start=False, stop=False)
        nc.tensor.matmul(po[:, HALF:D], sacc[:, 0:128], w0_t[:],
                         start=False, stop=True)

        if last:
            nc.vector.tensor_scalar_max(ob[:, HALF:D], po[:, HALF:D], 0.0)
        else:
            nc.scalar.activation(ob[:, HALF:D], po[:, HALF:D], RELU)
        nc.scalar.dma_start(out_d[r, HALF:D], ob[:, HALF:D])


def build_nc(ngroups=NG):
    bh = ngroups * GROUP
    nc = bacc.Bacc("TRN2", target_bir_lowering=False, debug=False)
    t0x = nc.dram_tensor("t0", [bh, NF], BF16, kind="ExternalInput")
    t1 = nc.dram_tensor("t1", [bh, NF], FP8, kind="ExternalInput")
    xst = nc.dram_tensor("xst", [F, bh], BF16, kind="ExternalInput")
    w_s = nc.dram_tensor("w_s", [F, HALF], BF16, kind="ExternalInput")
    w0 = nc.dram_tensor("w0", [F, HALF], BF16, kind="ExternalInput")
    w1 = nc.dram_tensor("w1", [F, HALF], BF16, kind="ExternalInput")
    bvec = nc.dram_tensor("bvec", [1, D], BF16, kind="ExternalInput")
    ident_d = nc.dram_tensor("ident", [128, 128], BF16, kind="ExternalInput")
    ones_d = nc.dram_tensor("ones", [1, 128], BF16, kind="ExternalInput")
    out = nc.dram_tensor("out", [bh, D], BF16, kind="ExternalOutput")

    ins = [t.ap() for t in (t0x, t1, xst, w_s, w0, w1, bvec, ident_d,
                            ones_d)]
    with nc.allow_low_precision("2e-2 rel-err budget admits fp8/bf16 path"):
        with tile.TileContext(nc) as tc:
            _tile_kernel(tc, [out.ap()], ins, ngroups)
    nc.compile()
    return nc


def make_in_maps(x_self, x_neigh_0, x_neigh_1, w_self, w_neigh_0, w_neigh_1, b):
    """Shard full inputs into per-core input maps (batch axis, 8 ways).

    Host-side mixed-precision cast: xn0/x_self/weights -> bf16, xn1 ->
    fp8-e4m3.  The 2e-2 tolerance admits it and it cuts the HBM traffic
    of this memory-bound kernel ~2.6x.  xn0 and x_self are packed into
    one row-major tensor so each row group is two large DMAs per ring.
    """
    xs16 = np.asarray(x_self, dtype=np.float32).astype(BF16NP)
    xn0_16 = np.asarray(x_neigh_0, dtype=np.float32).astype(BF16NP)
    xn1_8 = np.asarray(x_neigh_1, dtype=np.float32).astype(FP8NP)
    scale = np.float32(1.0 / (N * NR))
    w_s = np.asarray(w_self, dtype=np.float32).astype(BF16NP)
    w0 = (np.asarray(w_neigh_0, dtype=np.float32) * scale).astype(BF16NP)
    w1 = (np.asarray(w_neigh_1, dtype=np.float32) * scale).astype(BF16NP)
    bvec = np.asarray(b, dtype=np.float32).reshape(1, D).astype(BF16NP)
    ident = np.eye(128, dtype=np.float32).astype(BF16NP)
    ones = np.ones((1, 128), dtype=np.float32).astype(BF16NP)

    t0 = xn0_16.reshape(B * H, NF)
    t1 = xn1_8.reshape(B * H, NF)
    xst = np.ascontiguousarray(xs16.reshape(B * H, F).T)  # [F, B*H]

    in_maps = []
    for c in range(NCORES):
        rs = slice(c * BH, (c + 1) * BH)
        in_maps.append({
            "t0": np.ascontiguousarray(t0[rs]),
            "t1": np.ascontiguousarray(t1[rs]),
            "xst": np.ascontiguousarray(xst[:, rs]),
            "w_s": w_s, "w0": w0, "w1": w1, "bvec": bvec,
            "ident": ident, "ones": ones,
        })
    return in_maps


_NC_CACHE = None


def kernel(x_self, x_neigh_0, x_neigh_1, w_self, w_neigh_0, w_neigh_1, b):
    global _NC_CACHE
    if _NC_CACHE is None:
        _NC_CACHE = build_nc()
    in_maps = make_in_maps(x_self, x_neigh_0, x_neigh_1,
                           w_self, w_neigh_0, w_neigh_1, b)
    res = bass_utils.run_bass_kernel_spmd(
        _NC_CACHE, in_maps, core_ids=list(range(NCORES)))
    out = np.concatenate([r["out"] for r in res.results], axis=0)
    return out.astype(np.float32).reshape(B, H, D)


# revision 26
# speedup vs baseline: 1.0192x; 1.0192x over previous
"""Trainium2 Bass kernel: MeanHinAggregator (GNN message passing).

Reference computation (per batch-head element bh):
    z_r  = mean_n(x_neigh_r[bh, n, :]) @ w_neigh_r          (r = 0, 1)
    out  = relu(concat(x_self[bh] @ w_self, (z0 + z1) / 2) + b)

Strategy (pure data parallel over 8 NeuronCores, batch axis sharded):
  * The 2e-2 relative-error budget admits aggressive mixed precision.
    x_self / weights / output are bf16; one neighbour tensor (xn1) is
    cast to fp8-e4m3 on the host - its mean over 32 samples averages the
    quantization noise down, and the self projection dominates the output
    norm.  Measured end-to-end rel-err vs the fp32 reference: ~4e-3.
    Per-core HBM traffic drops 44.2 -> 16.7 MB.  (Going fp8 on BOTH
    neighbour tensors is a net loss: fp8 reads run the DVE folds at 1x -
    no 8-bit packing on TRN2 - so the kernel turns compute-bound.)
  * Per core: B_shard=128, H=10 -> 1280 rows, processed in 10 groups of
    128 rows.  Three dram tensors: xn0 bf16 [1280, 4096], xn1 fp8
    [1280, 4096], and x_self^T bf16 [128, 1280] (host-pre-transposed so
    the self projection reads it directly as lhsT - no transpose matmul,
    PSUM block, or copy; it is loaded once as a single 0.33 MB DMA).
    Each group issues two ~0.5 MiB DMAs per HWDGE ring, byte-balanced
    across rings, t1 pieces first so the slower fp8 fold starts
    earliest.  Output stores ride the ACT ring right after ReLU.
  * Mean over the 32 neighbour slices: in-place bf16 tree-folds on the
    Vector engine for xn0 (2x_1P DVE mode) and an fp8->bf16 first level
    plus a bf16 level for xn1, each down to 8 slices; eight accumulating
    transposing matmuls per tensor (lhsT = slice, rhs = identity) finish
    the sum on the PE while transposing into the [f, bh] layout the
    projection needs as lhsT.  This balances DVE (~4.9us) against DMA
    (~5us) and PE (~4us) per group.
  * Projection: out[bh, d] = sumT.T @ w with the 1/(N*NR) scaling folded
    into host-prescaled bf16 copies of w_neigh_*.  Bias is added with a
    K=1 matmul accumulating into PSUM.  PSUM -> SBUF copies are split per
    128-column block on the Scalar engine; the xn0-dependent projection
    matmul (its folds finish last) closes each group.  The first and
    last groups fold each DMA'd half independently, shortening the
    pipeline ramp and the post-last-DMA serial tail.
"""

import numpy as np
import ml_dtypes

import concourse.bacc as bacc
import concourse.bass as bass
import concourse.tile as tile
from concourse import bass_utils, mybir
from concourse._compat import with_exitstack

B, H, N, F = 1024, 10, 32, 128
HALF = 128
D = 2 * HALF
NR = 2
NCORES = 8
BSH = B // NCORES        # 128 batch rows per core
BH = BSH * H             # 1280 (bh rows per core)
GROUP = 128              # bh rows per group
NG = BH // GROUP         # 10 groups
NF = N * F               # 4096
LOOKAHEAD = 4            # groups of DMA prefetch beyond the current one
F32 = mybir.dt.float32
BF16 = mybir.dt.bfloat16
FP8 = mybir.dt.float8e4
BF16NP = np.dtype(ml_dtypes.bfloat16)
FP8NP = np.dtype(ml_dtypes.float8_e4m3)
RELU = mybir.ActivationFunctionType.Relu
COPY = mybir.ActivationFunctionType.Copy


@with_exitstack
def _tile_kernel(ctx, tc, outs, ins, ngroups):
    nc = tc.nc
    t0_d, t1_d, xst_d, w_s, w0, w1, bvec, ident_d, ones_d = ins
    (out_d,) = outs

    const = ctx.enter_context(tc.tile_pool(name="const", bufs=1))
    xpool = ctx.enter_context(tc.tile_pool(name="xp", bufs=LOOKAHEAD + 1))
    fpool = ctx.enter_context(tc.tile_pool(name="fp", bufs=4))
    spool = ctx.enter_context(tc.tile_pool(name="sp", bufs=4))
    opool = ctx.enter_context(tc.tile_pool(name="op", bufs=4))
    ppool = ctx.enter_context(tc.tile_pool(name="ps", bufs=3, space="PSUM"))
    pout = ctx.enter_context(tc.tile_pool(name="po", bufs=3, space="PSUM"))

    def issue_loads(g):
        r = slice(g * GROUP, (g + 1) * GROUP)
        t0 = xpool.tile([128, NF], BF16, tag="t0")
        t1 = xpool.tile([128, NF], FP8, tag="t1")
        nc.sync.dma_start(t1[:, 0:2048], t1_d[r, 0:2048])
        nc.scalar.dma_start(t1[:, 2048:NF], t1_d[r, 2048:NF])
        nc.sync.dma_start(t0[:, 0:2048], t0_d[r, 0:2048])
        nc.scalar.dma_start(t0[:, 2048:NF], t0_d[r, 2048:NF])
        return t0, t1

    pending = [issue_loads(0)]

    # All of x_self^T for this core in one contiguous load (2.5 KiB per
    # partition); the per-group projection just slices its columns.
    xst = const.tile([128, BH], BF16, tag="xst")
    nc.scalar.dma_start(xst[:], xst_d[:])

    ident = const.tile([128, 128], BF16, tag="ident")
    nc.sync.dma_start(ident[:], ident_d[:])
    wS_t = const.tile([128, HALF], BF16, tag="wS")
    nc.sync.dma_start(wS_t[:], w_s[:])
    w0_t = const.tile([128, HALF], BF16, tag="w0")
    nc.sync.dma_start(w0_t[:], w0[:])
    w1_t = const.tile([128, HALF], BF16, tag="w1")
    nc.sync.dma_start(w1_t[:], w1[:])
    b_t = const.tile([1, D], BF16, tag="b")
    nc.sync.dma_start(b_t[:], bvec[:])
    ones_t = const.tile([1, 128], BF16, tag="ones")
    nc.sync.dma_start(ones_t[:], ones_d[:])

    for g in range(1, min(LOOKAHEAD, ngroups)):
        pending.append(issue_loads(g))

    def transpose_accum(pacc, col, f, slices):
        for i, c in enumerate(slices):
            nc.tensor.matmul(pacc[:, col:col + 128], f[:, c:c + F], ident[:],
                             start=(i == 0), stop=(i == len(slices) - 1))

    SL8 = tuple(i * F for i in range(8))

    for g in range(ngroups):
        r = slice(g * GROUP, (g + 1) * GROUP)
        t0, t1 = pending.pop(0)
        if g + LOOKAHEAD < ngroups:
            pending.append(issue_loads(g + LOOKAHEAD))
        edge = g == 0 or g == ngroups - 1

        # xn1: fp8 first level (1x on DVE) into a bf16 tile, one bf16
        # level, leaving 8 slices.  The first and last groups fold each
        # DMA'd half independently (halves land on different rings), so
        # the ramp starts earlier and the serial tail is shorter.
        f1 = fpool.tile([128, 2048], BF16, tag="f1")
        if edge:
            nc.vector.tensor_add(f1[:, 0:1024], t1[:, 0:1024],
                                 t1[:, 1024:2048])
            nc.vector.tensor_add(f1[:, 1024:2048], t1[:, 2048:3072],
                                 t1[:, 3072:NF])
            nc.vector.tensor_add(f1[:, 0:512], f1[:, 0:512], f1[:, 512:1024])
            nc.vector.tensor_add(f1[:, 1024:1536], f1[:, 1024:1536],
                                 f1[:, 1536:2048])
        else:
            nc.vector.tensor_add(f1[:], t1[:, 0:2048], t1[:, 2048:NF])
            nc.vector.tensor_add(f1[:, 0:1024], f1[:, 0:1024],
                                 f1[:, 1024:2048])

        # xn0: in-place bf16 levels (2x_1P), leaving 8 slices.
        if edge:
            nc.vector.tensor_add(t0[:, 0:1024], t0[:, 0:1024],
                                 t0[:, 1024:2048])
            nc.vector.tensor_add(t0[:, 2048:3072], t0[:, 2048:3072],
                                 t0[:, 3072:NF])
            nc.vector.tensor_add(t0[:, 0:512], t0[:, 0:512], t0[:, 512:1024])
            nc.vector.tensor_add(t0[:, 2048:2560], t0[:, 2048:2560],
                                 t0[:, 2560:3072])
        else:
            nc.vector.tensor_add(t0[:, 0:2048], t0[:, 0:2048], t0[:, 2048:NF])
            nc.vector.tensor_add(t0[:, 0:1024], t0[:, 0:1024],
                                 t0[:, 1024:2048])

        sl1 = (0, F, 2 * F, 3 * F, 1024, 1024 + F, 1024 + 2 * F,
               1024 + 3 * F) if edge else SL8
        sl0 = (0, F, 2 * F, 3 * F, 2048, 2048 + F, 2048 + 2 * F,
               2048 + 3 * F) if edge else SL8

        # pacc[:, 0:128] = sum_n xn0 (as [f, bh]), [:, 128:256] = sum_n
        # xn1; accumulating transposing matmuls.  x_self needs no
        # transpose: it is DMA'd pre-transposed ([f, bh]) from the host.
        pacc = ppool.tile([128, 2 * 128], F32, tag="pacc")
        transpose_accum(pacc, 128, f1, sl1)
        transpose_accum(pacc, 0, t0, sl0)

        # PSUM -> SBUF (bf16): xn1 block first, xn0 block (whose folds
        # finish last) second, so early projections don't wait.
        sacc = spool.tile([128, 2 * 128], BF16, tag="sacc")
        nc.scalar.activation(sacc[:, 128:256], pacc[:, 128:256], COPY)
        nc.scalar.activation(sacc[:, 0:128], pacc[:, 0:128], COPY)

        # Projection: out[bh, d]; bias broadcast via K=1 matmuls; the
        # xn0-dependent matmul (its folds finish last) closes each group.
        po = pout.tile([128, D], F32, tag="po")
        nc.tensor.matmul(po[:, 0:HALF], ones_t[:], b_t[:, 0:HALF],
                         start=True, stop=False)
        nc.tensor.matmul(po[:, 0:HALF], xst[:, r], wS_t[:],
                         start=False, stop=True)
        nc.tensor.matmul(po[:, HALF:D], ones_t[:], b_t[:, HALF:D],
                         start=True, stop=False)
        nc.tensor.matmul(po[:, HALF:D], sacc[:, 128:256], w1_t[:],
                         start=False, stop=False)
        nc.tensor.matmul(po[:, HALF:D], sacc[:, 0:128], w0_t[:],
                         start=False, stop=True)

        ob = opool.tile([128, D], BF16, tag="ob")
        nc.scalar.activation(ob[:], po[:], RELU)
        nc.scalar.dma_start(out_d[r, :], ob[:])


def build_nc(ngroups=NG):
    bh = ngroups * GROUP
    nc = bacc.Bacc("TRN2", target_bir_lowering=False, debug=False)
    t0x = nc.dram_tensor("t0", [bh, NF], BF16, kind="ExternalInput")
    t1 = nc.dram_tensor("t1", [bh, NF], FP8, kind="ExternalInput")
    xst = nc.dram_tensor("xst", [F, bh], BF16, kind="ExternalInput")
    w_s = nc.dram_tensor("w_s", [F, HALF], BF16, kind="ExternalInput")
    w0 = nc.dram_tensor("w0", [F, HALF], BF16, kind="ExternalInput")
    w1 = nc.dram_tensor("w1", [F, HALF], BF16, kind="ExternalInput")
    bvec = nc.dram_tensor("bvec", [1, D], BF16, kind="ExternalInput")
    ident_d = nc.dram_tensor("ident", [128, 128], BF16, kind="ExternalInput")
    ones_d = nc.dram_tensor("ones", [1, 128], BF16, kind="ExternalInput")
    out = nc.dram_tensor("out", [bh, D], BF16, kind="ExternalOutput")

    ins = [t.ap() for t in (t0x, t1, xst, w_s, w0, w1, bvec, ident_d,
                            ones_d)]
    with nc.allow_low_precision("2e-2 rel-err budget admits fp8/bf16 path"):
        with tile.TileContext(nc) as tc:
            _tile_kernel(tc, [out.ap()], ins, ngroups)
    nc.compile()
    return nc


def make_in_maps(x_self, x_neigh_0, x_neigh_1, w_self, w_neigh_0, w_neigh_1, b):
    """Shard full inputs into per-core input maps (batch axis, 8 ways).

    Host-side mixed-precision cast: xn0/x_self/weights -> bf16, xn1 ->
    fp8-e4m3.  The 2e-2 tolerance admits it and it cuts the HBM traffic
    of this memory-bound kernel ~2.6x.  xn0 and x_self are packed into
    one row-major tensor so each row group is two large DMAs per ring.
    """
    xs16 = np.asarray(x_self, dtype=np.float32).astype(BF16NP)
    xn0_16 = np.asarray(x_neigh_0, dtype=np.float32).astype(BF16NP)
    xn1_8 = np.asarray(x_neigh_1, dtype=np.float32).astype(FP8NP)
    scale = np.float32(1.0 / (N * NR))
    w_s = np.asarray(w_self, dtype=np.float32).astype(BF16NP)
    w0 = (np.asarray(w_neigh_0, dtype=np.float32) * scale).astype(BF16NP)
    w1 = (np.asarray(w_neigh_1, dtype=np.float32) * scale).astype(BF16NP)
    bvec = np.asarray(b, dtype=np.float32).reshape(1, D).astype(BF16NP)
    ident = np.eye(128, dtype=np.float32).astype(BF16NP)
    ones = np.ones((1, 128), dtype=np.float32).astype(BF16NP)

    t0 = xn0_16.reshape(B * H, NF)
    t1 = xn1_8.reshape(B * H, NF)
    xst = np.ascontiguousarray(xs16.reshape(B * H, F).T)  # [F, B*H]

    in_maps = []
    for c in range(NCORES):
        rs = slice(c * BH, (c + 1) * BH)
        in_maps.append({
            "t0": np.ascontiguousarray(t0[rs]),
            "t1": np.ascontiguousarray(t1[rs]),
            "xst": np.ascontiguousarray(xst[:, rs]),
            "w_s": w_s, "w0": w0, "w1": w1, "bvec": bvec,
            "ident": ident, "ones": ones,
        })
    return in_maps


_NC_CACHE = None


def kernel(x_self, x_neigh_0, x_neigh_1, w_self, w_neigh_0, w_neigh_1, b):
    global _NC_CACHE
    if _NC_CACHE is None:
        _NC_CACHE = build_nc()
    in_maps = make_in_maps(x_self, x_neigh_0, x_neigh_1,
                           w_self, w_neigh_0, w_neigh_1, b)
    res = bass_utils.run_bass_kernel_spmd(
        _NC_CACHE, in_maps, core_ids=list(range(NCORES)))
    out = np.concatenate([r["out"] for r in res.results], axis=0)
    return out.astype(np.float32).reshape(B, H, D)


# revision 27
# speedup vs baseline: 1.0861x; 1.0656x over previous
"""Trainium2 Bass kernel: MeanHinAggregator (GNN message passing).

Reference computation (per batch-head element bh):
    z_r  = mean_n(x_neigh_r[bh, n, :]) @ w_neigh_r          (r = 0, 1)
    out  = relu(concat(x_self[bh] @ w_self, (z0 + z1) / 2) + b)

Strategy (pure data parallel over 8 NeuronCores, batch axis sharded):
  * The 2e-2 relative-error budget admits aggressive mixed precision.
    x_self / weights / output are bf16; one neighbour tensor (xn1) is
    cast to fp8-e4m3 on the host - its mean over 32 samples averages the
    quantization noise down, and the self projection dominates the output
    norm.  Measured end-to-end rel-err vs the fp32 reference: ~4e-3.
    Per-core HBM traffic drops 44.2 -> 16.7 MB.  (Going fp8 on BOTH
    neighbour tensors is a net loss: fp8 reads run the DVE folds at 1x -
    no 8-bit packing on TRN2 - so the kernel turns compute-bound.)
  * Per core: B_shard=128, H=10 -> 1280 rows, processed in 10 groups of
    128 rows.  Three dram tensors: xn0 bf16 [1280, 4096], xn1 fp8
    [1280, 4096], and x_self^T bf16 [128, 1280] (host-pre-transposed so
    the self projection reads it directly as lhsT - no transpose matmul,
    PSUM block, or copy; it is loaded once as a single 0.33 MB DMA).
    Each group issues two ~0.5 MiB DMAs per HWDGE ring, byte-balanced
    across rings, t1 pieces first so the slower fp8 fold starts
    earliest.  Output stores ride the ACT ring right after ReLU.
  * Mean over the 32 neighbour slices: in-place bf16 tree-folds on the
    Vector engine for xn0 (2x_1P DVE mode) and an fp8->bf16 first level
    plus a bf16 level for xn1, each down to 8 slices; eight accumulating
    transposing matmuls per tensor (lhsT = slice, rhs = identity) finish
    the sum on the PE while transposing into the [f, bh] layout the
    projection needs as lhsT.  This balances DVE (~4.9us) against DMA
    (~5us) and PE (~4us) per group.
  * Projection: out[bh, d] = sumT.T @ w with the 1/(N*NR) scaling folded
    into host-prescaled bf16 copies of w_neigh_*.  Bias is added with a
    K=1 matmul accumulating into PSUM.  PSUM -> SBUF copies are split per
    128-column block on the Scalar engine; the xn0-dependent projection
    matmul (its folds finish last) closes each group.  The first and
    last groups fold each DMA'd half independently, shortening the
    pipeline ramp and the post-last-DMA serial tail.
"""

import numpy as np
import ml_dtypes

import concourse.bacc as bacc
import concourse.bass as bass
import concourse.tile as tile
from concourse import bass_utils, mybir
from concourse._compat import with_exitstack

B, H, N, F = 1024, 10, 32, 128
HALF = 128
D = 2 * HALF
NR = 2
NCORES = 8
BSH = B // NCORES        # 128 batch rows per core
BH = BSH * H             # 1280 (bh rows per core)
GROUP = 128              # bh rows per group
NG = BH // GROUP         # 10 groups
NF = N * F               # 4096
LOOKAHEAD = 4            # groups of DMA prefetch beyond the current one
F32 = mybir.dt.float32
BF16 = mybir.dt.bfloat16
FP8 = mybir.dt.float8e4
BF16NP = np.dtype(ml_dtypes.bfloat16)
FP8NP = np.dtype(ml_dtypes.float8_e4m3)
RELU = mybir.ActivationFunctionType.Relu
COPY = mybir.ActivationFunctionType.Copy


@with_exitstack
def _tile_kernel(ctx, tc, outs, ins, ngroups):
    nc = tc.nc
    t0_d, t1_d, xst_d, w_s, w0, w1, bvec, ident_d, ones_d = ins
    (out_d,) = outs

    const = ctx.enter_context(tc.tile_pool(name="const", bufs=1))
    xpool = ctx.enter_context(tc.tile_pool(name="xp", bufs=LOOKAHEAD + 1))
    fpool = ctx.enter_context(tc.tile_pool(name="fp", bufs=4))
    spool = ctx.enter_context(tc.tile_pool(name="sp", bufs=4))
    opool = ctx.enter_context(tc.tile_pool(name="op", bufs=4))
    ppool = ctx.enter_context(tc.tile_pool(name="ps", bufs=3, space="PSUM"))
    pout = ctx.enter_context(tc.tile_pool(name="po", bufs=3, space="PSUM"))

    def issue_loads(g):
        r = slice(g * GROUP, (g + 1) * GROUP)
        t0 = xpool.tile([128, NF], BF16, tag="t0")
        t1 = xpool.tile([128, NF], FP8, tag="t1")
        if g == 0 or g == ngroups - 1:
            # Edge groups: halves across rings for the per-half folds.
            nc.sync.dma_start(t1[:, 0:2048], t1_d[r, 0:2048])
            nc.scalar.dma_start(t1[:, 2048:NF], t1_d[r, 2048:NF])
            nc.sync.dma_start(t0[:, 0:2048], t0_d[r, 0:2048])
            nc.scalar.dma_start(t0[:, 2048:NF], t0_d[r, 2048:NF])
        elif g % 2 == 0:
            nc.scalar.dma_start(t1[:], t1_d[r, :])
            nc.sync.dma_start(t0[:], t0_d[r, :])
        else:
            nc.sync.dma_start(t1[:], t1_d[r, :])
            nc.scalar.dma_start(t0[:], t0_d[r, :])
        return t0, t1

    pending = [issue_loads(0)]

    # All of x_self^T for this core in one contiguous load (2.5 KiB per
    # partition); the per-group projection just slices its columns.
    xst = const.tile([128, BH], BF16, tag="xst")
    nc.scalar.dma_start(xst[:], xst_d[:])

    ident = const.tile([128, 128], BF16, tag="ident")
    nc.sync.dma_start(ident[:], ident_d[:])
    wS_t = const.tile([128, HALF], BF16, tag="wS")
    nc.sync.dma_start(wS_t[:], w_s[:])
    w0_t = const.tile([128, HALF], BF16, tag="w0")
    nc.sync.dma_start(w0_t[:], w0[:])
    w1_t = const.tile([128, HALF], BF16, tag="w1")
    nc.sync.dma_start(w1_t[:], w1[:])
    b_t = const.tile([1, D], BF16, tag="b")
    nc.sync.dma_start(b_t[:], bvec[:])
    ones_t = const.tile([1, 128], BF16, tag="ones")
    nc.sync.dma_start(ones_t[:], ones_d[:])

    for g in range(1, min(LOOKAHEAD, ngroups)):
        pending.append(issue_loads(g))

    def transpose_accum(pacc, col, f, slices):
        for i, c in enumerate(slices):
            nc.tensor.matmul(pacc[:, col:col + 128], f[:, c:c + F], ident[:],
                             start=(i == 0), stop=(i == len(slices) - 1))

    SL8 = tuple(i * F for i in range(8))

    for g in range(ngroups):
        r = slice(g * GROUP, (g + 1) * GROUP)
        t0, t1 = pending.pop(0)
        if g + LOOKAHEAD < ngroups:
            pending.append(issue_loads(g + LOOKAHEAD))
        edge = g == 0 or g == ngroups - 1

        # xn1: fp8 first level (1x on DVE) into a bf16 tile, one bf16
        # level, leaving 8 slices.  The first and last groups fold each
        # DMA'd half independently (halves land on different rings), so
        # the ramp starts earlier and the serial tail is shorter.
        f1 = fpool.tile([128, 2048], BF16, tag="f1")
        if edge:
            nc.vector.tensor_add(f1[:, 0:1024], t1[:, 0:1024],
                                 t1[:, 1024:2048])
            nc.vector.tensor_add(f1[:, 1024:2048], t1[:, 2048:3072],
                                 t1[:, 3072:NF])
            nc.vector.tensor_add(f1[:, 0:512], f1[:, 0:512], f1[:, 512:1024])
            nc.vector.tensor_add(f1[:, 1024:1536], f1[:, 1024:1536],
                                 f1[:, 1536:2048])
            nc.vector.tensor_add(f1[:, 0:256], f1[:, 0:256], f1[:, 256:512])
            nc.vector.tensor_add(f1[:, 1024:1280], f1[:, 1024:1280],
                                 f1[:, 1280:1536])
        else:
            nc.vector.tensor_add(f1[:], t1[:, 0:2048], t1[:, 2048:NF])
            nc.vector.tensor_add(f1[:, 0:1024], f1[:, 0:1024],
                                 f1[:, 1024:2048])

        # xn0: in-place bf16 levels (2x_1P), leaving 8 slices.
        if edge:
            nc.vector.tensor_add(t0[:, 0:1024], t0[:, 0:1024],
                                 t0[:, 1024:2048])
            nc.vector.tensor_add(t0[:, 2048:3072], t0[:, 2048:3072],
                                 t0[:, 3072:NF])
            nc.vector.tensor_add(t0[:, 0:512], t0[:, 0:512], t0[:, 512:1024])
            nc.vector.tensor_add(t0[:, 2048:2560], t0[:, 2048:2560],
                                 t0[:, 2560:3072])
            nc.vector.tensor_add(t0[:, 0:256], t0[:, 0:256], t0[:, 256:512])
            nc.vector.tensor_add(t0[:, 2048:2304], t0[:, 2048:2304],
                                 t0[:, 2304:2560])
        else:
            nc.vector.tensor_add(t0[:, 0:2048], t0[:, 0:2048], t0[:, 2048:NF])
            nc.vector.tensor_add(t0[:, 0:1024], t0[:, 0:1024],
                                 t0[:, 1024:2048])

        sl1 = (0, F, 1024, 1024 + F) if edge else SL8
        sl0 = (0, F, 2048, 2048 + F) if edge else SL8

        # pacc[:, 0:128] = sum_n xn0 (as [f, bh]), [:, 128:256] = sum_n
        # xn1; accumulating transposing matmuls.  x_self needs no
        # transpose: it is DMA'd pre-transposed ([f, bh]) from the host.
        pacc = ppool.tile([128, 2 * 128], F32, tag="pacc")
        transpose_accum(pacc, 128, f1, sl1)
        transpose_accum(pacc, 0, t0, sl0)

        # PSUM -> SBUF (bf16): xn1 block first, xn0 block (whose folds
        # finish last) second, so early projections don't wait.
        sacc = spool.tile([128, 2 * 128], BF16, tag="sacc")
        nc.scalar.activation(sacc[:, 128:256], pacc[:, 128:256], COPY)
        nc.scalar.activation(sacc[:, 0:128], pacc[:, 0:128], COPY)

        # Projection: out[bh, d]; bias broadcast via K=1 matmuls; the
        # xn0-dependent matmul (its folds finish last) closes each group.
        po = pout.tile([128, D], F32, tag="po")
        nc.tensor.matmul(po[:, 0:HALF], ones_t[:], b_t[:, 0:HALF],
                         start=True, stop=False)
        nc.tensor.matmul(po[:, 0:HALF], xst[:, r], wS_t[:],
                         start=False, stop=True)
        nc.tensor.matmul(po[:, HALF:D], ones_t[:], b_t[:, HALF:D],
                         start=True, stop=False)
        nc.tensor.matmul(po[:, HALF:D], sacc[:, 128:256], w1_t[:],
                         start=False, stop=False)
        nc.tensor.matmul(po[:, HALF:D], sacc[:, 0:128], w0_t[:],
                         start=False, stop=True)

        ob = opool.tile([128, D], BF16, tag="ob")
        nc.scalar.activation(ob[:], po[:], RELU)
        nc.scalar.dma_start(out_d[r, :], ob[:])


def build_nc(ngroups=NG):
    bh = ngroups * GROUP
    nc = bacc.Bacc("TRN2", target_bir_lowering=False, debug=False)
    t0x = nc.dram_tensor("t0", [bh, NF], BF16, kind="ExternalInput")
    t1 = nc.dram_tensor("t1", [bh, NF], FP8, kind="ExternalInput")
    xst = nc.dram_tensor("xst", [F, bh], BF16, kind="ExternalInput")
    w_s = nc.dram_tensor("w_s", [F, HALF], BF16, kind="ExternalInput")
    w0 = nc.dram_tensor("w0", [F, HALF], BF16, kind="ExternalInput")
    w1 = nc.dram_tensor("w1", [F, HALF], BF16, kind="ExternalInput")
    bvec = nc.dram_tensor("bvec", [1, D], BF16, kind="ExternalInput")
    ident_d = nc.dram_tensor("ident", [128, 128], BF16, kind="ExternalInput")
    ones_d = nc.dram_tensor("ones", [1, 128], BF16, kind="ExternalInput")
    out = nc.dram_tensor("out", [bh, D], BF16, kind="ExternalOutput")

    ins = [t.ap() for t in (t0x, t1, xst, w_s, w0, w1, bvec, ident_d,
                            ones_d)]
    with nc.allow_low_precision("2e-2 rel-err budget admits fp8/bf16 path"):
        with tile.TileContext(nc) as tc:
            _tile_kernel(tc, [out.ap()], ins, ngroups)
    nc.compile()
    return nc


def make_in_maps(x_self, x_neigh_0, x_neigh_1, w_self, w_neigh_0, w_neigh_1, b):
    """Shard full inputs into per-core input maps (batch axis, 8 ways).

    Host-side mixed-precision cast: xn0/x_self/weights -> bf16, xn1 ->
    fp8-e4m3.  The 2e-2 tolerance admits it and it cuts the HBM traffic
    of this memory-bound kernel ~2.6x.  xn0 and x_self are packed into
    one row-major tensor so each row group is two large DMAs per ring.
    """
    xs16 = np.asarray(x_self, dtype=np.float32).astype(BF16NP)
    xn0_16 = np.asarray(x_neigh_0, dtype=np.float32).astype(BF16NP)
    xn1_8 = np.asarray(x_neigh_1, dtype=np.float32).astype(FP8NP)
    scale = np.float32(1.0 / (N * NR))
    w_s = np.asarray(w_self, dtype=np.float32).astype(BF16NP)
    w0 = (np.asarray(w_neigh_0, dtype=np.float32) * scale).astype(BF16NP)
    w1 = (np.asarray(w_neigh_1, dtype=np.float32) * scale).astype(BF16NP)
    bvec = np.asarray(b, dtype=np.float32).reshape(1, D).astype(BF16NP)
    ident = np.eye(128, dtype=np.float32).astype(BF16NP)
    ones = np.ones((1, 128), dtype=np.float32).astype(BF16NP)

    t0 = xn0_16.reshape(B * H, NF)
    t1 = xn1_8.reshape(B * H, NF)
    xst = np.ascontiguousarray(xs16.reshape(B * H, F).T)  # [F, B*H]

    in_maps = []
    for c in range(NCORES):
        rs = slice(c * BH, (c + 1) * BH)
        in_maps.append({
            "t0": np.ascontiguousarray(t0[rs]),
            "t1": np.ascontiguousarray(t1[rs]),
            "xst": np.ascontiguousarray(xst[:, rs]),
            "w_s": w_s, "w0": w0, "w1": w1, "bvec": bvec,
            "ident": ident, "ones": ones,
        })
    return in_maps


_NC_CACHE = None


def kernel(x_self, x_neigh_0, x_neigh_1, w_self, w_neigh_0, w_neigh_1, b):
    global _NC_CACHE
    if _NC_CACHE is None:
        _NC_CACHE = build_nc()
    in_maps = make_in_maps(x_self, x_neigh_0, x_neigh_1,
                           w_self, w_neigh_0, w_neigh_1, b)
    res = bass_utils.run_bass_kernel_spmd(
        _NC_CACHE, in_maps, core_ids=list(range(NCORES)))
    out = np.concatenate([r["out"] for r in res.results], axis=0)
    return out.astype(np.float32).reshape(B, H, D)
